# revision 23
# baseline (speedup 1.0000x reference)
"""Binary Conv2d (sign-act 3x3 binary conv + RPReLU + residual) on 8 trn2 NeuronCores.

Reference computation (forward values):
  a  = sign(x + move0_bias)                       # {-1,0,+1}
  bw = scale_o * sign(conv_w), scale_o = mean |conv_w| over (I,KH,KW)
  z  = conv2d(a, bw, pad=1) + pr_bias0
  y  = where(z>=0, z, alpha*z) + pr_bias1 + x

Strategy: data-parallel over batch (16 imgs -> 2 per core). Conv as 9 tap
matmuls with fp8e4 DoubleRow (contracts both 128-channel chunks per matmul,
2 MACs/cell/cycle) accumulating in PSUM; activations are exact sign values
in fp8, stored in a zero-bordered 66-wide padded tile per (img); weights are
sign(w) fp8 (exact).

Epilogue (exact in f32, valid for alpha < 1): with z = s*p + b0,
  y = PReLU(z) + b1 + x = [(p*s + x) + (b0+b1)] + Relu(-(1-a)*s*p - (1-a)*b0)
i.e. one ACT Relu (PSUM->SBUF) plus two fused DVE scalar_tensor_tensor ops:
  t1 = (p * s) + x ; y = (t1 + (b0+b1)) + ut.   GpSimd does no epilogue work.

Startup: a dummy Sign on a const tile pulls the ~1.3us ACT table load off the
critical path; the weight DMA rides the Scalar HWDGE queue (oc0 half first)
so it doesn't fair-share with the x transfers on the Sync queue; 12 warm-up
matmuls release the PE HAM clock gate (1.2 -> 2.4 GHz) during the DMA window.
The kernel ends on two single-block units so the final non-overlappable
epilogue chain is half-length.
"""

import sys
for _p in ("/opt/trn_rl_repo",):
    if _p not in sys.path:
        sys.path.append(_p)

from contextlib import ExitStack

import numpy as np
import ml_dtypes

import concourse.bass as bass
import concourse.tile as tile
from concourse import bacc, mybir
from concourse import bass_utils

N_CORES = 8
B, C, H, W = 16, 256, 64, 64
K = 3
BPC = B // N_CORES            # imgs per core
NCH = C // 128                # channel chunks (2)
PW = W + 2                    # padded width 66
PHR = 72                      # padded rows allocated (>=66, CST 16-aligned)
CST = PHR * PW                # per-chunk stride in act tile (4752, %16==0)
SP = H * W                    # spatial 4096
RB = 8                        # out rows per block
NBLK = H // RB                # 8 blocks
NBE = RB * W                  # 512 block elems
NTAP = K * K
WHALF = NTAP * NCH * 128      # weight cols per oc chunk (2304)

F32 = mybir.dt.float32
BF16 = mybir.dt.bfloat16
FP8 = mybir.dt.float8e4

import os
N_WARMUP = int(os.environ.get("K_N_WARMUP", "10"))
STT2_GPS = os.environ.get("K_STT2_GPS", "1") == "1"

_CACHE = {}


def _build_program():
    nc = bacc.Bacc(
        "TRN2",
        target_bir_lowering=False,
        debug=False,
        enable_asserts=False,
        num_devices=N_CORES,
    )
    # x ships as bf16: halves DMA bytes; sign() is exact on bf16 (rounding
    # never flips a sign) and the residual's ~0.1% rounding is far inside
    # the 2e-2 gate
    x_d = nc.dram_tensor("x", [BPC, C, H, W], BF16, kind="ExternalInput").ap()
    # weight pack: [128, oc(2) * tap(9) * icpair(2) * 128] fp8 sign values
    w_d = nc.dram_tensor("w", [128, NCH * WHALF], FP8,
                         kind="ExternalInput").ap()
    mb_d = nc.dram_tensor("mb", [C, 1], F32, kind="ExternalInput").ap()
    # epilogue constants: [C, 4] = [s, b0+b1, -(1-a)s, -(1-a)b0]
    epi_d = nc.dram_tensor("epi", [C, 4], F32, kind="ExternalInput").ap()
    y_d = nc.dram_tensor("y", [BPC, C, H, W], F32, kind="ExternalOutput").ap()

    with tile.TileContext(nc) as tc:
        _kernel(tc, y_d, x_d, w_d, mb_d, epi_d)
    nc.compile()
    return nc


def _kernel(tc, y_d, x_d, w_d, mb_d, epi_d):
    nc = tc.nc
    ctx = ExitStack()
    with ctx:
        const = ctx.enter_context(tc.tile_pool(name="const", bufs=1))
        xpool = ctx.enter_context(tc.tile_pool(name="x", bufs=1))
        apool = ctx.enter_context(tc.tile_pool(name="act", bufs=1))
        work = ctx.enter_context(tc.tile_pool(name="work", bufs=4))
        psum = ctx.enter_context(tc.tile_pool(name="psum", bufs=4, space="PSUM"))

        # --- tiles ---
        x_flat = x_d.rearrange("b c h w -> b c (h w)")
        y_flat = y_d.rearrange("b c h w -> b c (h w)")
        xt = {}   # (b, ic) -> [128, 4096] f32 (residual source)
        at = {}   # b -> [128, 2*CST] fp8 padded sign, chunk ic at offset ic*CST
        for b in range(BPC):
            at[b] = apool.tile([128, NCH * CST], FP8, tag=f"at{b}",
                               name=f"at{b}")
            for ic in range(NCH):
                xt[b, ic] = xpool.tile([128, SP], BF16, tag=f"xt{b}{ic}",
                                       name=f"xt{b}{ic}")
        wt = const.tile([128, NCH * WHALF], FP8, tag="wt")
        mbt = [const.tile([128, 1], F32, tag=f"mb{ic}", name=f"mbt{ic}")
               for ic in range(NCH)]
        ept = [const.tile([128, 4], F32, tag=f"ep{oc}", name=f"ept{oc}")
               for oc in range(NCH)]
        dsig_in = const.tile([128, 1], F32, tag="dsi", name="dsig_in")
        dsig_out = const.tile([128, 1], F32, tag="dso", name="dsig_out")

        # --- PE warm-up: dummy matmuls release the HAM clock gate; they run
        # during the startup DMA window and end roughly when data is ready
        warm = const.tile([128, NBE], FP8, tag="warm")
        nc.gpsimd.memset(warm[:], 1.0)
        wps = psum.tile([128, 2 * NBE], F32, tag="pt", name="wps")
        for _ in range(N_WARMUP):
            nc.tensor.matmul(wps[:, 0:NBE], warm[:, 0:128], warm[:],
                             start=True, stop=True)

        # --- DMA plan. The Sync HWDGE stripes packets of every queued
        # transfer round-robin, so concurrent transfers dilute each other.
        # Critical path (img0 groups 0-1, both chunks) rides the faster
        # Scalar HWDGE queue; the weights ride the GpSimd queue; img1 is
        # four big transfers issued last on Sync so img0's groups 2-7 get
        # the Sync bandwidth while the matmul stream consumes them.
        GROUPS = [(0, 10), (10, 18), (18, 26), (26, 34), (34, 42), (42, 50),
                  (50, 58), (58, 64)]
        G1 = [(0, 32), (32, 64)]

        def dma_x_rows(q, b, ic, r0, r1):
            xs = xt[b, ic][:, r0 * W:r1 * W]
            return q.dma_start(
                out=xs,
                in_=x_flat[b, ic * 128:(ic + 1) * 128, r0 * W:r1 * W])

        # dummy Sign pulls the ~1.3us ACT table load off the critical path;
        # then the Scalar queue issues g0's transfers and goes on to signs
        nc.vector.memset(dsig_in[:], 0.0)
        nc.scalar.activation(dsig_out[:], dsig_in[:],
                             mybir.ActivationFunctionType.Sign,
                             bias=0.0, scale=1.0)
        dma_x_rows(nc.scalar, 0, 0, *GROUPS[0])
        dma_x_rows(nc.scalar, 0, 1, *GROUPS[0])

        # img0's borders zero on the otherwise-idle Vector engine so they
        # gate neither DMA-issue queue (they're needed by the first matmul)
        def zero_borders(eng, b):
            a4 = at[b][:].rearrange("p (i h w) -> p i h w", i=NCH, w=PW)
            eng.memset(a4[:, :, 0:1, :], 0.0)
            eng.memset(a4[:, :, H + 1:H + 2, :], 0.0)
            eng.memset(a4[:, :, 1:H + 1, 0:1], 0.0)
            eng.memset(a4[:, :, 1:H + 1, PW - 1:PW], 0.0)

        zero_borders(nc.vector, 0)

        # weights + g1 ride the GpSimd SWDGE queue (warm memset stays first
        # so the PE warm-up starts immediately). g1c0 leads: the first
        # matmul's dependency is coarsened to include sign(g1) by subtile
        # granularity, so g1 is on the critical path too.
        dma_x_rows(nc.gpsimd, 0, 0, *GROUPS[1])
        nc.gpsimd.dma_start(out=wt[:, 0:WHALF], in_=w_d[:, 0:WHALF])
        dma_x_rows(nc.gpsimd, 0, 1, *GROUPS[1])
        nc.gpsimd.dma_start(out=wt[:, WHALF:], in_=w_d[:, WHALF:])
        zero_borders(nc.gpsimd, 1)

        for ic in range(NCH):
            nc.sync.dma_start(out=mbt[ic][:],
                              in_=mb_d[ic * 128:(ic + 1) * 128])
        for oc in range(NCH):
            nc.sync.dma_start(out=ept[oc][:],
                              in_=epi_d[oc * 128:(oc + 1) * 128])
        for g in range(2, len(GROUPS)):
            for ic in range(NCH):
                dma_x_rows(nc.sync, 0, ic, *GROUPS[g])
        # img1's x transfers are NOT issued here: they are emitted between
        # the b0 and b1 conv loops so the early DMA window belongs to img0

        def emit_signs(b):
            # per DMA row group: each sign fires as soon as its rows land
            for (r0, r1) in GROUPS:
                for ic in range(NCH):
                    xs = xt[b, ic][:, r0 * W:r1 * W]
                    a4 = at[b][:].rearrange("p (i h w) -> p i h w",
                                            i=NCH, w=PW)
                    x3 = xs.rearrange("p (h w) -> p h w", w=W)
                    nc.scalar.activation(
                        a4[:, ic, 1 + r0:1 + r1, 1:1 + W], x3,
                        mybir.ActivationFunctionType.Sign,
                        bias=mbt[ic][:], scale=1.0)

        # --- conv blocks: pairs of 8-row blocks share a 2-bank PSUM tile;
        # the very last work is two half blocks so the final epilogue
        # chain (which cannot overlap any matmul) is quarter-length.
        # Signs for image b are emitted just before image b's blocks so
        # the ACT queue never head-of-line blocks epilogue Relus behind
        # signs whose DMA hasn't landed. ---
        NPAIR = NBLK // 2
        PBE = 2 * NBE            # 1024 elems per pair
        gunit = 0                # global unit index
        for b in range(BPC):
            if b == 1:
                # issue img1's x transfers now (sync engine reaches these
                # after b0oc0's y DMAs, ~20us in, once img0 is delivered)
                for g in range(len(G1)):
                    for ic in range(NCH):
                        dma_x_rows(nc.sync, 1, ic, *G1[g])
            emit_signs(b)
            a4 = at[b][:].rearrange("p (i h w) -> p i h w", i=NCH, w=PW)
            for oc in range(NCH):
                final_grp = (b == BPC - 1 and oc == NCH - 1)
                if final_grp:
                    units = [(0, 16), (16, 16), (32, 16), (48, 8),
                             (56, 4), (60, 4)]
                else:
                    units = [(pr * 16, 16) for pr in range(NPAIR)]
                for ui, (r0u, nru) in enumerate(units):
                    ube = nru * W
                    pt = psum.tile([128, PBE], F32, tag="pt")
                    # split the unit's rows into <=8-row matmul banks
                    off = 0
                    for rb0 in range(r0u, r0u + nru, RB):
                        nr = min(RB, r0u + nru - rb0)
                        out_half = pt[:, off:off + nr * W]
                        off += nr * W
                        for kh in range(K):
                            for kw in range(K):
                                t = kh * K + kw
                                wsl = wt[:, (oc * NTAP + t) * NCH
                                         * 128:(oc * NTAP + t + 1) * NCH * 128]
                                lhsT = wsl.rearrange("p (i m) -> p i m", i=NCH)
                                rhs = a4[:, :, rb0 + kh:rb0 + kh + nr,
                                         kw:kw + W]
                                nc.tensor.matmul(
                                    out_half, lhsT, rhs,
                                    start=(t == 0), stop=(t == NTAP - 1),
                                    perf_mode=mybir.MatmulPerfMode.DoubleRow)
                    # epilogue on the unit; (b0+b1) is pre-folded into x
                    # host-side, so:
                    #   ut = Relu(-(1-a)*s*p - (1-a)*b0)     [ACT]
                    #   t1 = (p * s) + x'                    [DVE stt]
                    #   y  = t1 + ut                         [DVE or GpSimd]
                    ep = ept[oc]
                    base = r0u * W
                    sl = slice(0, ube)
                    xsl = xt[b, oc][:, base:base + ube]
                    ut = work.tile([128, PBE], F32, tag="ut", name="ut")
                    nc.scalar.activation(
                        ut[:, sl], pt[:, sl],
                        mybir.ActivationFunctionType.Relu,
                        bias=ep[:, 3:4], scale=ep[:, 2:3])
                    t1 = work.tile([128, PBE], F32, tag="t1", name="t1")
                    nc.vector.scalar_tensor_tensor(
                        out=t1[:, sl], in0=pt[:, sl], scalar=ep[:, 0:1],
                        in1=xsl, op0=mybir.AluOpType.mult,
                        op1=mybir.AluOpType.add)
                    yt = work.tile([128, PBE], F32, tag="yt", name="yt")
                    # mid-stream units' final add goes to the otherwise-idle
                    # GpSimd to keep DVE from backlogging at stream end; the
                    # latency-critical first and last units stay on DVE
                    # (GpSimd is ~3x slower per element, so keep it clear of
                    # the final (b,oc) group entirely)
                    eng = (nc.gpsimd if STT2_GPS and 4 <= gunit <= 13
                           else nc.vector)
                    eng.tensor_add(out=yt[:, sl], in0=t1[:, sl],
                                   in1=ut[:, sl])
                    # the last units' outputs alternate Scalar/Sync so the
                    # final issue+drain parallelizes across both queues
                    if final_grp and ui >= len(units) - 3:
                        yq = nc.sync if ui == len(units) - 2 else nc.scalar
                    else:
                        yq = nc.sync
                    yq.dma_start(
                        out=y_flat[b, oc * 128:(oc + 1) * 128,
                                   base:base + ube],
                        in_=yt[:, sl])
                    gunit += 1


def _pack_inputs(x, move0_bias, conv_w, pr_bias0, prelu_alpha, pr_bias1):
    """Host-side prep: weight binarization + epilogue constant folding."""
    f32 = np.float32
    w = conv_w.astype(f32)
    scale = np.abs(w).mean(axis=(1, 2, 3)).astype(f32)          # (O,)
    ws = np.sign(w).astype(ml_dtypes.float8_e4m3)               # (O,I,KH,KW)
    # lhsT[k=p, oc, tap, ic, m] = ws[oc*128+m, ic*128+p, kh, kw]
    wsr = ws.reshape(NCH, 128, NCH, 128, NTAP)                  # (oc,m,ic,p,t)
    lhsT = wsr.transpose(3, 0, 4, 2, 1)                         # (p,oc,t,ic,m)
    lhsT = np.ascontiguousarray(lhsT).reshape(128, NCH * WHALF)

    alpha = prelu_alpha.astype(f32).reshape(C)
    b0 = pr_bias0.astype(f32).reshape(C)
    b1 = pr_bias1.astype(f32).reshape(C)
    assert np.all(alpha < 1.0)
    epi = np.stack([scale, np.zeros_like(scale),
                    -(1 - alpha) * scale, -(1 - alpha) * b0],
                   axis=1).astype(f32)
    # fold c = b0+b1 into x (residual path) and subtract it from the sign
    # bias so sign(x' + mb') == sign(x + mb) exactly
    c = (b0 + b1).astype(f32)
    mb = move0_bias.astype(f32).reshape(C, 1) - c.reshape(C, 1)
    xp = (x.astype(f32) + c.reshape(1, C, 1, 1)).astype(ml_dtypes.bfloat16)

    common = {"w": lhsT, "mb": mb, "epi": epi}
    in_maps = []
    for i in range(N_CORES):
        m = dict(common)
        m["x"] = np.ascontiguousarray(xp[i * BPC:(i + 1) * BPC])
        in_maps.append(m)
    return in_maps


def kernel(x, move0_bias, conv_w, pr_bias0, prelu_alpha, pr_bias1):
    in_maps = _pack_inputs(
        np.asarray(x), np.asarray(move0_bias), np.asarray(conv_w),
        np.asarray(pr_bias0), np.asarray(prelu_alpha), np.asarray(pr_bias1))
    if "nc" not in _CACHE:
        _CACHE["nc"] = _build_program()
    nc = _CACHE["nc"]
    res = bass_utils.run_bass_kernel_spmd(nc, in_maps,
                                          core_ids=list(range(N_CORES)))
    _CACHE["last_results"] = res
    out = np.concatenate([res.results[i]["y"] for i in range(N_CORES)], axis=0)
    return out
